# revision 22
# baseline (speedup 1.0000x reference)
"""Trainium2 Bass kernel for a batched linear-chain CRF negative log-likelihood.

reference semantics (B=128, S=2048, T=128):
    forward algorithm over S steps (log-space matvec chain) -> log_Z per batch
    gold path score = emissions gathered at tags + transitions gathered at
    (tag_t, tag_{t+1}) pairs, summed over time
    output = mean(log_Z - seq_score)   (scalar f32)

Strategy:
  - data parallel over 8 cores: 16 batch rows per core, transitions replicated.
  - linear space: a_t = (a_{t-1} @ W) * E_t with W = exp(transitions),
    E_t = exp(emit_t - chat).  Per-step work: one PE matmul (stationary W,
    moving state [128 tags x 16 batch]) + one DVE multiply out of PSUM.
  - bidirectional: forward chain from t=0 and a backward chain
    y_t = E_t * (W @ y_{t+1}) from t=2047 run concurrently and meet at
    t=1023: log_Z = log(a_m . (W y_{m+1})) + accumulated log scales.
  - renormalization every 32 steps; colsum scale logs parked and ln'd once
    in the epilogue.
  - E precomputed in a pre-phase into a transposed [tag, b*S+t] bf16 buffer
    via PE transpose + scalar-engine exp evacuation (bias = -chat).
  - gold path in the same pre-phase, via one fp32 matmul per (b, sblock):
    CD_b += OH^T @ [OHshift | EMIS]  (N=256).  The left half accumulates the
    tag-pair count matrix, the right half accumulates D[i,j] = sum_s
    OH[s,i] e[s,j] whose diagonal is the emission-select sum.  Finalized per
    batch row with one elementwise multiply by [trans | identity] and a
    grouped reduce.
"""

import numpy as np

B, S, T = 128, 2048, 128
NCORES = 8
BC = B // NCORES  # 16 batch rows per core
NSB = S // 128  # 16 s-blocks of 128
MID = S // 2 - 1  # 1023: chains meet here
RENORM = 32
JUNK_TAG = 60000.0  # one-hot of this is all zeros (tags are < 128)

_compiled = None


def _build_program(do_chain=True, do_gold=True, nrot=None):
    import concourse.bass as bass
    import concourse.bacc as bacc
    import concourse.tile as tile
    from concourse import mybir
    from concourse.masks import make_identity

    fp32 = mybir.dt.float32
    bf16 = mybir.dt.bfloat16
    AF = mybir.ActivationFunctionType
    ALU = mybir.AluOpType
    AX = mybir.AxisListType

    nc = bacc.Bacc(None)
    em_d = nc.declare_dram_parameter("emissions_sh", [BC, S, T], fp32, isOutput=False)
    tr_d = nc.declare_dram_parameter("transitions", [T, T], fp32, isOutput=False)
    tg_d = nc.declare_dram_parameter("tags_sh", [BC, S], mybir.dt.int32, isOutput=False)
    out_d = nc.declare_dram_parameter("loss_parts", [BC], fp32, isOutput=True)

    with tile.TileContext(nc) as tc:
        with (
            tc.tile_pool(name="consts", bufs=1) as consts,
            tc.tile_pool(name="ebuf", bufs=1) as ebufp,
            tc.tile_pool(name="emis", bufs=4) as emisp,
            tc.tile_pool(name="oh", bufs=4) as ohp,
            tc.tile_pool(name="dump", bufs=4) as dumpp,
            tc.tile_pool(name="state", bufs=4) as statep,
            tc.tile_pool(name="small", bufs=6) as smallp,
            tc.tile_pool(name="tp_ps", bufs=2, space="PSUM") as tp_ps,
            tc.tile_pool(name="q_ps", bufs=4, space="PSUM") as q_ps,
            tc.tile_pool(name="cd_ps", bufs=1, space="PSUM") as cd_ps,
            tc.tile_pool(name="m_ps", bufs=1, space="PSUM") as m_ps,
        ):
            # ---------------- constants ----------------
            ident = consts.tile([128, 128], fp32)
            make_identity(nc, ident)
            ident_bf = consts.tile([128, 128], bf16)
            make_identity(nc, ident_bf)
            iota = consts.tile([128, 128], fp32)
            nc.gpsimd.iota(
                iota, pattern=[[1, 128]], base=0, channel_multiplier=0,
                allow_small_or_imprecise_dtypes=True,
            )
            ones_col_bf = consts.tile([128, 1], bf16)
            nc.vector.memset(ones_col_bf, 1.0)
            ones_col_f = consts.tile([128, 1], fp32)
            nc.vector.memset(ones_col_f, 1.0)
            ones_row_f = consts.tile([1, 128], fp32)
            nc.vector.memset(ones_row_f, 1.0)

            # transitions -> W = exp(trans) bf16, WT = W^T bf16
            tr_sb = consts.tile([128, 128], fp32)
            nc.sync.dma_start(out=tr_sb, in_=tr_d[:, :])
            w_bf = consts.tile([128, 128], bf16)
            nc.scalar.activation(w_bf, tr_sb, AF.Exp)
            wt_psum = tp_ps.tile([128, 128], bf16, tag="tp")
            nc.tensor.transpose(wt_psum, w_bf, ident_bf)
            wt_bf = consts.tile([128, 128], bf16)
            nc.vector.tensor_copy(wt_bf, wt_psum)

            # [trans | identity] for the gold finalize
            tri = consts.tile([128, 256], fp32)
            nc.vector.tensor_copy(tri[:, 0:128], tr_sb)
            nc.vector.tensor_copy(tri[:, 128:256], ident)

            # chat = mean_j ln(colsum_j W) over j=1..127  (col 0 is exp(-1e4)=0)
            colw_ps = m_ps.tile([1, 128], fp32, tag="m")
            nc.tensor.matmul(colw_ps, ones_col_bf, w_bf, start=True, stop=True)
            lncol = smallp.tile([1, 127], fp32, tag="lncol")
            lnsum = consts.tile([1, 1], fp32)
            nc.scalar.activation(lncol, colw_ps[:, 1:128], AF.Ln, accum_out=lnsum)
            chat_tot = consts.tile([1, 1], fp32)
            nc.scalar.activation(chat_tot, lnsum, AF.Copy, scale=float(S) / 127.0)
            negchat = consts.tile([1, 1], fp32)
            nc.scalar.activation(negchat, lnsum, AF.Copy, scale=-1.0 / 127.0)
            nbc_ps = m_ps.tile([128, 1], fp32, tag="m")
            nc.tensor.matmul(nbc_ps, ones_row_f, negchat, start=True, stop=True)
            negchat_bc = consts.tile([128, 1], fp32)
            nc.vector.tensor_copy(negchat_bc, nbc_ps)

            # tags -> f32, transposed into [s(128), (sb,b)] column layout,
            # plus a shift-by-one variant for transition pairs
            tags_nat = consts.tile([BC, S], mybir.dt.int32)
            nc.sync.dma_start(out=tags_nat, in_=tg_d[:, :])
            tags_f = consts.tile([BC, S], fp32)
            nc.vector.tensor_copy(tags_f, tags_nat)
            tag_cols = consts.tile([128, NSB * BC], fp32)   # col = sb*16 + b
            tagsh_cols = consts.tile([128, NSB * BC], fp32)
            nc.vector.memset(tagsh_cols[:, (NSB - 1) * BC:], JUNK_TAG)
            for sb in range(NSB):
                tp = tp_ps.tile([128, BC], fp32, tag="tp")
                nc.tensor.transpose(
                    tp, tags_f[:, sb * 128:(sb + 1) * 128], ident[:BC, :BC]
                )
                nc.vector.tensor_copy(tag_cols[:, sb * BC:(sb + 1) * BC], tp)
            for sb in range(NSB):
                n = 128 if sb < NSB - 1 else 127
                tp = tp_ps.tile([128, BC], fp32, tag="tp")
                nc.tensor.transpose(
                    tp[:n], tags_f[:, sb * 128 + 1: sb * 128 + 1 + n],
                    ident[:BC, :BC],
                )
                nc.vector.tensor_copy(
                    tagsh_cols[:n, sb * BC:(sb + 1) * BC], tp[:n]
                )

            # ---------------- pre-phase: gold + E precompute ----------------
            ebuf = ebufp.tile([128, S * BC], bf16)  # free index = b*S + t
            ebuf3 = ebuf.rearrange("p (b t) -> p b t", t=S)
            # per-b [sum(C*trans) | esel] results: cols [2b, 2b+1]
            gsum = consts.tile([128, 2 * BC], fp32)

            def emit_E(b, sb):
                emis = emisp.tile([128, 128], fp32, tag="emis")
                nc.sync.dma_start(
                    out=emis, in_=em_d[b, sb * 128:(sb + 1) * 128, :]
                )
                tp = tp_ps.tile([128, 128], fp32, tag="tp")
                nc.tensor.transpose(tp, emis, ident)
                # exp(x - chat), contiguous run: free = b*S + sb*128 + s
                nc.scalar.activation(
                    ebuf3[:, b, sb * 128:(sb + 1) * 128], tp, AF.Exp,
                    bias=negchat_bc,
                )

            gold_cd = [None]

            def emit_gold(b, sb):
                col = sb * BC + b
                oh = ohp.tile([128, 128], fp32, tag="oh")
                nc.vector.tensor_scalar(
                    out=oh, in0=iota, scalar1=tag_cols[:, col:col + 1],
                    scalar2=None, op0=ALU.is_equal,
                )
                # rhs = [OHshift | EMIS]
                pair = ohp.tile([128, 256], fp32, tag="pair")
                nc.vector.tensor_scalar(
                    out=pair[:, 0:128], in0=iota,
                    scalar1=tagsh_cols[:, col:col + 1],
                    scalar2=None, op0=ALU.is_equal,
                )
                nc.sync.dma_start(
                    out=pair[:, 128:256], in_=em_d[b, sb * 128:(sb + 1) * 128, :]
                )
                if sb == 0:
                    gold_cd[0] = cd_ps.tile(
                        [128, 256], fp32, tag="cd", name="gold_cd"
                    )
                nc.tensor.matmul(
                    gold_cd[0], oh, pair, start=(sb == 0), stop=(sb == NSB - 1)
                )
                if sb == NSB - 1:
                    # finalize row b: [C|D] * [trans|ident], grouped reduce
                    cdump = dumpp.tile([128, 256], fp32, tag="cdump")
                    nc.vector.tensor_tensor(
                        out=cdump, in0=gold_cd[0], in1=tri, op=ALU.mult
                    )
                    nc.vector.tensor_reduce(
                        gsum[:, 2 * b:2 * b + 2],
                        cdump.rearrange("p (c j) -> p c j", c=2),
                        axis=AX.X, op=ALU.add,
                    )

            if do_gold:
                for b in range(BC):
                    for sb in range(NSB):
                        emit_gold(b, sb)
            else:
                nc.vector.memset(gsum, 0.0)
            for sb in range(NSB):
                for b in range(BC):
                    emit_E(b, sb)

            # ---------------- chain ----------------
            NRE = 64
            glog = consts.tile([1, BC * NRE], fp32)
            nc.vector.memset(glog, 1.0)
            glog3 = glog.rearrange("p (b k) -> p b k", k=NRE)
            renorm_k = [0]

            def renorm(v):
                """colsum -> reciprocal -> broadcast; park colsum for epilogue."""
                cs = m_ps.tile([1, BC], fp32, tag="m")
                nc.tensor.matmul(cs, ones_col_bf, v, start=True, stop=True)
                rec = smallp.tile([1, BC], fp32, tag="rec")
                nc.vector.reciprocal(rec, cs)
                k = renorm_k[0]
                renorm_k[0] += 1
                nc.vector.tensor_copy(glog3[:, :, k], cs)
                bc_ps = m_ps.tile([128, BC], fp32, tag="m")
                nc.tensor.matmul(bc_ps, ones_row_f, rec, start=True, stop=True)
                return bc_ps

            def eslice(t):
                return ebuf3[:, :, t]

            vf = eslice(0)          # a_0 = E_0
            vb = eslice(S - 1)      # y_{2047} = E_{2047}
            bc_f = None
            bc_b = None
            vb_fin = None
            NROT = S - 1 - MID      # 1024 rotations
            nrot_lim = NROT if nrot is None else nrot
            for r in range(NROT if do_chain else 0):
                if r >= nrot_lim:
                    break
                # forward step t = r+1:  a_t = (a_{t-1} @ W) * E_t  (lhsT=W)
                if r < MID:
                    t = r + 1
                    qf = q_ps.tile([128, BC], fp32, tag="q")
                    nc.tensor.matmul(qf, w_bf, vf, start=True, stop=True)
                    nvf = statep.tile([128, BC], bf16, tag="vf")
                    nc.vector.tensor_tensor(out=nvf, in0=qf, in1=eslice(t), op=ALU.mult)
                    if bc_f is not None:
                        nc.vector.tensor_tensor(out=nvf, in0=nvf, in1=bc_f, op=ALU.mult)
                        bc_f = None
                    vf = nvf
                    if (t % RENORM == 0 or t == 1008) and t < MID:
                        bc_f = renorm(vf)
                # backward: q = W @ y_{t+1}; t from 2046 down to MID
                t = S - 2 - r
                qb = q_ps.tile([128, BC], fp32, tag="q")
                nc.tensor.matmul(qb, wt_bf, vb, start=True, stop=True)
                if t == MID:
                    vb_fin = qb  # b_MID = W y_{MID+1}: final, stays in PSUM
                else:
                    nvb = statep.tile([128, BC], bf16, tag="vb")
                    nc.vector.tensor_tensor(out=nvb, in0=qb, in1=eslice(t), op=ALU.mult)
                    if bc_b is not None:
                        nc.vector.tensor_tensor(out=nvb, in0=nvb, in1=bc_b, op=ALU.mult)
                        bc_b = None
                    vb = nvb
                    # scale from a renorm at t applies at step t-1; last chance
                    # is t == MID+2
                    if (t % RENORM == 0 or t == 1040) and t > MID + 1:
                        bc_b = renorm(vb)

            if not do_chain or nrot_lim < NROT:
                vvf = statep.tile([128, BC], bf16, tag="vf")
                nc.vector.memset(vvf, 1.0)
                vf = vvf
                vb_fin = q_ps.tile([128, BC], fp32, tag="q", name="vbfin")
                nc.tensor.matmul(vb_fin, wt_bf, vvf, start=True, stop=True)

            # ---------------- epilogue ----------------
            # log_Z = ln(sum_j vf*vb_fin) + sum(ln renorm scales) + S*chat
            dotd = dumpp.tile([128, BC], fp32, tag="dotd")
            nc.vector.tensor_tensor(out=dotd, in0=vb_fin, in1=vf, op=ALU.mult)
            zs = m_ps.tile([1, BC], fp32, tag="m")
            nc.tensor.matmul(zs, ones_col_f, dotd, start=True, stop=True)
            lnz = smallp.tile([1, BC], fp32, tag="lnz")
            nc.scalar.activation(lnz, zs, AF.Ln)
            lnglog = smallp.tile([1, BC * NRE], fp32, tag="lnglog")
            nc.scalar.activation(lnglog, glog, AF.Ln)
            accsum = smallp.tile([1, BC], fp32, tag="accsum")
            nc.vector.tensor_reduce(
                accsum,
                lnglog.rearrange("p (b k) -> p b k", k=NRE),
                axis=AX.X, op=ALU.add,
            )
            logz = smallp.tile([1, BC], fp32, tag="logz")
            nc.vector.tensor_tensor(out=logz, in0=lnz, in1=accsum, op=ALU.add)
            nc.vector.tensor_scalar(
                out=logz, in0=logz, scalar1=chat_tot, scalar2=None, op0=ALU.add
            )

            # seq score from gsum columns: [2b] = sum(C*trans), [2b+1] = esel
            gs_ps = m_ps.tile([1, 2 * BC], fp32, tag="m")
            nc.tensor.matmul(gs_ps, ones_col_f, gsum, start=True, stop=True)
            res = smallp.tile([1, BC], fp32, tag="res")
            seq = gs_ps.rearrange("p (b c) -> p b c", c=2)
            nc.vector.tensor_tensor(out=res, in0=logz, in1=seq[:, :, 0], op=ALU.subtract)
            nc.vector.tensor_tensor(out=res, in0=res, in1=seq[:, :, 1], op=ALU.subtract)
            nc.sync.dma_start(out=out_d[:], in_=res[0:1, :])

    return nc


def _get_compiled(finalized=False):
    global _compiled
    if _compiled is None:
        _compiled = _build_program()
    if finalized and not _compiled.is_finalized():
        _compiled.finalize()
    return _compiled


def make_in_maps(emissions, transitions, tags):
    in_maps = []
    for c in range(NCORES):
        sl = slice(c * BC, (c + 1) * BC)
        in_maps.append({
            "emissions_sh": np.ascontiguousarray(emissions[sl], dtype=np.float32),
            "transitions": np.ascontiguousarray(transitions, dtype=np.float32),
            "tags_sh": np.ascontiguousarray(tags[sl]).astype(np.int32),
        })
    return in_maps


def _run_device(emissions, transitions, tags):
    from concourse.bass_utils import run_bass_kernel_spmd

    nc = _get_compiled(finalized=True)
    res = run_bass_kernel_spmd(
        nc, make_in_maps(emissions, transitions, tags), list(range(NCORES))
    )
    parts = np.concatenate([res.results[c]["loss_parts"] for c in range(NCORES)])
    return np.float32(parts.mean())


def _run_host(emissions, transitions, tags, mask):
    """Slow but fully general fallback (any mask pattern)."""
    e = emissions.astype(np.float64)
    t = transitions.astype(np.float64)

    def lse(x, axis):
        m = x.max(axis=axis, keepdims=True)
        return (m + np.log(np.exp(x - m).sum(axis=axis, keepdims=True))).squeeze(axis)

    score = e[:, 0]
    for s in range(1, e.shape[1]):
        nxt = lse(score[:, :, None] + t[None, :, :] + e[:, s, None, :], axis=1)
        score = np.where(mask[:, s, None], nxt, score)
    log_Z = lse(score, axis=1)
    emit = np.take_along_axis(e, tags[..., None].astype(np.int64), axis=2)[..., 0]
    trans_sc = t[tags[:, :-1].astype(np.int64), tags[:, 1:].astype(np.int64)]
    m = mask[:, 1:].astype(np.float64)
    seq = emit[:, 0] + ((trans_sc + emit[:, 1:]) * m).sum(axis=1)
    return np.float32((log_Z - seq).mean())


def kernel(emissions, transitions, tags, mask):
    emissions = np.asarray(emissions)
    transitions = np.asarray(transitions)
    tags = np.asarray(tags)
    mask = np.asarray(mask)
    if emissions.shape != (B, S, T) or not mask.all():
        return _run_host(emissions, transitions, tags, mask)
    return _run_device(emissions, transitions, tags)


# revision 23
# speedup vs baseline: 1.5552x; 1.5552x over previous
"""Trainium2 Bass kernel for a batched linear-chain CRF negative log-likelihood.

reference semantics (B=128, S=2048, T=128):
    forward algorithm over S steps (log-space matvec chain) -> log_Z per batch
    gold path score = emissions gathered at tags + transitions gathered at
    (tag_t, tag_{t+1}) pairs, summed over time
    output = mean(log_Z - seq_score)   (scalar f32)

Strategy:
  - data parallel over 8 cores: 16 batch rows per core, transitions replicated.
  - linear space: a_t = (a_{t-1} @ W) * E_t with W = exp(transitions),
    E_t = exp(emit_t - chat).  Per-step work: one PE matmul (stationary W,
    moving state [128 tags x 16 batch]) + one DVE multiply out of PSUM.
  - bidirectional: forward chain from t=0 and a backward chain
    y_t = E_t * (W @ y_{t+1}) from t=2047 run concurrently and meet at
    t=1023: log_Z = log(a_m . (W y_{m+1})) + accumulated log scales.
  - renormalization every 32 steps; colsum scale logs parked and ln'd once
    in the epilogue.
  - E precomputed in a pre-phase into a transposed [tag, b*S+t] bf16 buffer
    via PE transpose + scalar-engine exp evacuation (bias = -chat).
  - gold path in the same pre-phase, via one fp32 matmul per (b, sblock):
    CD_b += OH^T @ [OHshift | EMIS]  (N=256).  The left half accumulates the
    tag-pair count matrix, the right half accumulates D[i,j] = sum_s
    OH[s,i] e[s,j] whose diagonal is the emission-select sum.  Finalized per
    batch row with one elementwise multiply by [trans | identity] and a
    grouped reduce.
"""

import numpy as np

B, S, T = 128, 2048, 128
NCORES = 8
BC = B // NCORES  # 16 batch rows per core
NSB = S // 128  # 16 s-blocks of 128
MID = S // 2 - 1  # 1023: chains meet here
RENORM = 64
JUNK_TAG = 60000.0  # one-hot of this is all zeros (tags are < 128)

_compiled = None


def _build_program(do_chain=True, do_gold=True, nrot=None):
    import concourse.bass as bass
    import concourse.bacc as bacc
    import concourse.tile as tile
    from concourse import mybir
    from concourse.masks import make_identity

    fp32 = mybir.dt.float32
    bf16 = mybir.dt.bfloat16
    AF = mybir.ActivationFunctionType
    ALU = mybir.AluOpType
    AX = mybir.AxisListType

    nc = bacc.Bacc(None)
    em_d = nc.declare_dram_parameter("emissions_sh", [BC, S, T], fp32, isOutput=False)
    tr_d = nc.declare_dram_parameter("transitions", [T, T], fp32, isOutput=False)
    tg_d = nc.declare_dram_parameter("tags_sh", [BC, S], mybir.dt.int32, isOutput=False)
    out_d = nc.declare_dram_parameter("loss_parts", [BC], fp32, isOutput=True)

    with tile.TileContext(nc) as tc:
        with (
            tc.tile_pool(name="consts", bufs=1) as consts,
            tc.tile_pool(name="ebuf", bufs=1) as ebufp,
            tc.tile_pool(name="emis", bufs=4) as emisp,
            tc.tile_pool(name="oh", bufs=4) as ohp,
            tc.tile_pool(name="dump", bufs=4) as dumpp,
            tc.tile_pool(name="state", bufs=4) as statep,
            tc.tile_pool(name="small", bufs=6) as smallp,
            tc.tile_pool(name="tp_ps", bufs=2, space="PSUM") as tp_ps,
            tc.tile_pool(name="q_ps", bufs=4, space="PSUM") as q_ps,
            tc.tile_pool(name="cd_ps", bufs=1, space="PSUM") as cd_ps,
            tc.tile_pool(name="m_ps", bufs=1, space="PSUM") as m_ps,
        ):
            # ---------------- constants ----------------
            ident = consts.tile([128, 128], fp32)
            make_identity(nc, ident)
            ident_bf = consts.tile([128, 128], bf16)
            make_identity(nc, ident_bf)
            iota = consts.tile([128, 128], bf16)
            nc.gpsimd.iota(
                iota, pattern=[[1, 128]], base=0, channel_multiplier=0,
                allow_small_or_imprecise_dtypes=True,
            )
            ones_col_bf = consts.tile([128, 1], bf16)
            nc.vector.memset(ones_col_bf, 1.0)
            ones_col_f = consts.tile([128, 1], fp32)
            nc.vector.memset(ones_col_f, 1.0)
            ones_row_f = consts.tile([1, 128], fp32)
            nc.vector.memset(ones_row_f, 1.0)

            # transitions -> W = exp(trans) bf16, WT = W^T bf16
            tr_sb = consts.tile([128, 128], fp32)
            nc.sync.dma_start(out=tr_sb, in_=tr_d[:, :])
            w_bf = consts.tile([128, 128], bf16)
            nc.scalar.activation(w_bf, tr_sb, AF.Exp)
            wt_psum = tp_ps.tile([128, 128], bf16, tag="tp")
            nc.tensor.transpose(wt_psum, w_bf, ident_bf)
            wt_bf = consts.tile([128, 128], bf16)
            nc.vector.tensor_copy(wt_bf, wt_psum)

            # [trans | identity] for the gold finalize
            tri = consts.tile([128, 256], fp32)
            nc.vector.tensor_copy(tri[:, 0:128], tr_sb)
            nc.vector.tensor_copy(tri[:, 128:256], ident)

            # chat = mean_j ln(colsum_j W) over j=1..127  (col 0 is exp(-1e4)=0)
            colw_ps = m_ps.tile([1, 128], fp32, tag="m")
            nc.tensor.matmul(colw_ps, ones_col_bf, w_bf, start=True, stop=True)
            lncol = smallp.tile([1, 127], fp32, tag="lncol")
            lnsum = consts.tile([1, 1], fp32)
            nc.scalar.activation(lncol, colw_ps[:, 1:128], AF.Ln, accum_out=lnsum)
            chat_tot = consts.tile([1, 1], fp32)
            nc.scalar.activation(chat_tot, lnsum, AF.Copy, scale=float(S) / 127.0)
            negchat = consts.tile([1, 1], fp32)
            nc.scalar.activation(negchat, lnsum, AF.Copy, scale=-1.0 / 127.0)
            nbc_ps = m_ps.tile([128, 1], fp32, tag="m")
            nc.tensor.matmul(nbc_ps, ones_row_f, negchat, start=True, stop=True)
            negchat_bc = consts.tile([128, 1], fp32)
            nc.vector.tensor_copy(negchat_bc, nbc_ps)

            # tags -> f32, transposed into [s(128), (sb,b)] column layout,
            # plus a shift-by-one variant for transition pairs
            tags_nat = consts.tile([BC, S], mybir.dt.int32)
            nc.sync.dma_start(out=tags_nat, in_=tg_d[:, :])
            tags_f = consts.tile([BC, S], fp32)
            nc.vector.tensor_copy(tags_f, tags_nat)
            tag_cols = consts.tile([128, NSB * BC], fp32)   # col = sb*16 + b
            tagsh_cols = consts.tile([128, NSB * BC], fp32)
            nc.vector.memset(tagsh_cols[:, (NSB - 1) * BC:], JUNK_TAG)
            for sb in range(NSB):
                tp = tp_ps.tile([128, BC], fp32, tag="tp")
                nc.tensor.transpose(
                    tp, tags_f[:, sb * 128:(sb + 1) * 128], ident[:BC, :BC]
                )
                nc.vector.tensor_copy(tag_cols[:, sb * BC:(sb + 1) * BC], tp)
            for sb in range(NSB):
                n = 128 if sb < NSB - 1 else 127
                tp = tp_ps.tile([128, BC], fp32, tag="tp")
                nc.tensor.transpose(
                    tp[:n], tags_f[:, sb * 128 + 1: sb * 128 + 1 + n],
                    ident[:BC, :BC],
                )
                nc.vector.tensor_copy(
                    tagsh_cols[:n, sb * BC:(sb + 1) * BC], tp[:n]
                )

            # ---------------- pre-phase: gold + E precompute ----------------
            ebuf = ebufp.tile([128, S * BC], bf16)  # free index = b*S + t
            ebuf3 = ebuf.rearrange("p (b t) -> p b t", t=S)
            # per-b [sum(C*trans) | esel] results: cols [2b, 2b+1]
            gsum = consts.tile([128, 2 * BC], fp32)

            def emit_E(b, sb):
                emis = emisp.tile([128, 128], fp32, tag="emis")
                nc.sync.dma_start(
                    out=emis, in_=em_d[b, sb * 128:(sb + 1) * 128, :]
                )
                tp = tp_ps.tile([128, 128], fp32, tag="tp")
                nc.tensor.transpose(tp, emis, ident)
                # exp(x - chat), contiguous run: free = b*S + sb*128 + s
                nc.scalar.activation(
                    ebuf3[:, b, sb * 128:(sb + 1) * 128], tp, AF.Exp,
                    bias=negchat_bc,
                )

            gold_cd = [None]

            def emit_gold(b, sb):
                col = sb * BC + b
                oh = ohp.tile([128, 128], bf16, tag="oh")
                nc.vector.tensor_scalar(
                    out=oh, in0=iota, scalar1=tag_cols[:, col:col + 1],
                    scalar2=None, op0=ALU.is_equal,
                )
                # rhs = [OHshift | EMIS]
                pair = ohp.tile([128, 256], bf16, tag="pair")
                nc.vector.tensor_scalar(
                    out=pair[:, 0:128], in0=iota,
                    scalar1=tagsh_cols[:, col:col + 1],
                    scalar2=None, op0=ALU.is_equal,
                )
                emis2 = emisp.tile([128, 128], fp32, tag="emis2")
                nc.sync.dma_start(
                    out=emis2, in_=em_d[b, sb * 128:(sb + 1) * 128, :]
                )
                nc.scalar.activation(pair[:, 128:256], emis2, AF.Copy)
                if sb == 0:
                    gold_cd[0] = cd_ps.tile(
                        [128, 256], fp32, tag="cd", name="gold_cd"
                    )
                nc.tensor.matmul(
                    gold_cd[0], oh, pair, start=(sb == 0), stop=(sb == NSB - 1)
                )
                if sb == NSB - 1:
                    # finalize row b: [C|D] * [trans|ident], grouped reduce
                    cdump = dumpp.tile([128, 256], fp32, tag="cdump")
                    nc.vector.tensor_tensor(
                        out=cdump, in0=gold_cd[0], in1=tri, op=ALU.mult
                    )
                    nc.vector.tensor_reduce(
                        gsum[:, 2 * b:2 * b + 2],
                        cdump.rearrange("p (c j) -> p c j", c=2),
                        axis=AX.X, op=ALU.add,
                    )

            side = []
            order = [0, NSB - 1]
            for k in range(1, NSB // 2):
                order += [k, NSB - 1 - k]
            for sb in order[2:]:
                for b in range(BC):
                    side.append(("E", b, sb))
            if do_gold:
                for b in range(BC):
                    for sb in range(NSB):
                        side.append(("G", b, sb))
            else:
                nc.vector.memset(gsum, 0.0)
            for sb in order[:2]:
                for b in range(BC):
                    emit_E(b, sb)

            def do_side(n):
                for _ in range(n):
                    if side:
                        kind, b, sb = side.pop(0)
                        if kind == "E":
                            emit_E(b, sb)
                        else:
                            emit_gold(b, sb)

            # ---------------- chain ----------------
            NRE = 64
            glog = consts.tile([1, BC * NRE], fp32)
            nc.vector.memset(glog, 1.0)
            glog3 = glog.rearrange("p (b k) -> p b k", k=NRE)
            renorm_k = [0]

            def renorm(v):
                """colsum -> reciprocal -> broadcast; park colsum for epilogue."""
                cs = m_ps.tile([1, BC], fp32, tag="m")
                nc.tensor.matmul(cs, ones_col_bf, v, start=True, stop=True)
                rec = smallp.tile([1, BC], fp32, tag="rec")
                nc.vector.reciprocal(rec, cs)
                k = renorm_k[0]
                renorm_k[0] += 1
                nc.vector.tensor_copy(glog3[:, :, k], cs)
                bc_ps = m_ps.tile([128, BC], fp32, tag="m")
                nc.tensor.matmul(bc_ps, ones_row_f, rec, start=True, stop=True)
                return bc_ps

            def eslice(t):
                return ebuf3[:, :, t]

            vf = eslice(0)          # a_0 = E_0
            vb = eslice(S - 1)      # y_{2047} = E_{2047}
            bc_f = None
            bc_b = None
            vb_fin = None
            NROT = S - 1 - MID      # 1024 rotations
            nrot_lim = NROT if nrot is None else nrot
            for r in range(NROT if do_chain else 0):
                if r >= nrot_lim:
                    break
                # forward step t = r+1:  a_t = (a_{t-1} @ W) * E_t  (lhsT=W)
                if r < MID:
                    t = r + 1
                    qf = q_ps.tile([128, BC], fp32, tag="q")
                    nc.tensor.matmul(qf, w_bf, vf, start=True, stop=True)
                    nvf = statep.tile([128, BC], bf16, tag="vf")
                    nc.vector.tensor_tensor(out=nvf, in0=qf, in1=eslice(t), op=ALU.mult)
                    if bc_f is not None:
                        nc.vector.tensor_tensor(out=nvf, in0=nvf, in1=bc_f, op=ALU.mult)
                        bc_f = None
                    vf = nvf
                    if (t % RENORM == 0 or t == 1008) and t < MID:
                        bc_f = renorm(vf)
                # backward: q = W @ y_{t+1}; t from 2046 down to MID
                t = S - 2 - r
                qb = q_ps.tile([128, BC], fp32, tag="q")
                nc.tensor.matmul(qb, wt_bf, vb, start=True, stop=True)
                if t == MID:
                    vb_fin = qb  # b_MID = W y_{MID+1}: final, stays in PSUM
                else:
                    nvb = statep.tile([128, BC], bf16, tag="vb")
                    nc.vector.tensor_tensor(out=nvb, in0=qb, in1=eslice(t), op=ALU.mult)
                    if bc_b is not None:
                        nc.vector.tensor_tensor(out=nvb, in0=nvb, in1=bc_b, op=ALU.mult)
                        bc_b = None
                    vb = nvb
                    # scale from a renorm at t applies at step t-1; last chance
                    # is t == MID+2
                    if (t % RENORM == 0 or t == 1040) and t > MID + 1:
                        bc_b = renorm(vb)
                do_side(1 if r % 2 == 0 else 0)

            do_side(len(side))
            if not do_chain or nrot_lim < NROT:
                vvf = statep.tile([128, BC], bf16, tag="vf")
                nc.vector.memset(vvf, 1.0)
                vf = vvf
                vb_fin = q_ps.tile([128, BC], fp32, tag="q", name="vbfin")
                nc.tensor.matmul(vb_fin, wt_bf, vvf, start=True, stop=True)

            # ---------------- epilogue ----------------
            # log_Z = ln(sum_j vf*vb_fin) + sum(ln renorm scales) + S*chat
            dotd = dumpp.tile([128, BC], fp32, tag="dotd")
            nc.vector.tensor_tensor(out=dotd, in0=vb_fin, in1=vf, op=ALU.mult)
            zs = m_ps.tile([1, BC], fp32, tag="m")
            nc.tensor.matmul(zs, ones_col_f, dotd, start=True, stop=True)
            lnz = smallp.tile([1, BC], fp32, tag="lnz")
            nc.scalar.activation(lnz, zs, AF.Ln)
            lnglog = smallp.tile([1, BC * NRE], fp32, tag="lnglog")
            nc.scalar.activation(lnglog, glog, AF.Ln)
            accsum = smallp.tile([1, BC], fp32, tag="accsum")
            nc.vector.tensor_reduce(
                accsum,
                lnglog.rearrange("p (b k) -> p b k", k=NRE),
                axis=AX.X, op=ALU.add,
            )
            logz = smallp.tile([1, BC], fp32, tag="logz")
            nc.vector.tensor_tensor(out=logz, in0=lnz, in1=accsum, op=ALU.add)
            nc.vector.tensor_scalar(
                out=logz, in0=logz, scalar1=chat_tot, scalar2=None, op0=ALU.add
            )

            # seq score from gsum columns: [2b] = sum(C*trans), [2b+1] = esel
            gs_ps = m_ps.tile([1, 2 * BC], fp32, tag="m")
            nc.tensor.matmul(gs_ps, ones_col_f, gsum, start=True, stop=True)
            res = smallp.tile([1, BC], fp32, tag="res")
            seq = gs_ps.rearrange("p (b c) -> p b c", c=2)
            nc.vector.tensor_tensor(out=res, in0=logz, in1=seq[:, :, 0], op=ALU.subtract)
            nc.vector.tensor_tensor(out=res, in0=res, in1=seq[:, :, 1], op=ALU.subtract)
            nc.sync.dma_start(out=out_d[:], in_=res[0:1, :])

    return nc


def _get_compiled(finalized=False):
    global _compiled
    if _compiled is None:
        _compiled = _build_program()
    if finalized and not _compiled.is_finalized():
        _compiled.finalize()
    return _compiled


def make_in_maps(emissions, transitions, tags):
    in_maps = []
    for c in range(NCORES):
        sl = slice(c * BC, (c + 1) * BC)
        in_maps.append({
            "emissions_sh": np.ascontiguousarray(emissions[sl], dtype=np.float32),
            "transitions": np.ascontiguousarray(transitions, dtype=np.float32),
            "tags_sh": np.ascontiguousarray(tags[sl]).astype(np.int32),
        })
    return in_maps


def _run_device(emissions, transitions, tags):
    from concourse.bass_utils import run_bass_kernel_spmd

    nc = _get_compiled(finalized=True)
    res = run_bass_kernel_spmd(
        nc, make_in_maps(emissions, transitions, tags), list(range(NCORES))
    )
    parts = np.concatenate([res.results[c]["loss_parts"] for c in range(NCORES)])
    return np.float32(parts.mean())


def _run_host(emissions, transitions, tags, mask):
    """Slow but fully general fallback (any mask pattern)."""
    e = emissions.astype(np.float64)
    t = transitions.astype(np.float64)

    def lse(x, axis):
        m = x.max(axis=axis, keepdims=True)
        return (m + np.log(np.exp(x - m).sum(axis=axis, keepdims=True))).squeeze(axis)

    score = e[:, 0]
    for s in range(1, e.shape[1]):
        nxt = lse(score[:, :, None] + t[None, :, :] + e[:, s, None, :], axis=1)
        score = np.where(mask[:, s, None], nxt, score)
    log_Z = lse(score, axis=1)
    emit = np.take_along_axis(e, tags[..., None].astype(np.int64), axis=2)[..., 0]
    trans_sc = t[tags[:, :-1].astype(np.int64), tags[:, 1:].astype(np.int64)]
    m = mask[:, 1:].astype(np.float64)
    seq = emit[:, 0] + ((trans_sc + emit[:, 1:]) * m).sum(axis=1)
    return np.float32((log_Z - seq).mean())


def kernel(emissions, transitions, tags, mask):
    emissions = np.asarray(emissions)
    transitions = np.asarray(transitions)
    tags = np.asarray(tags)
    mask = np.asarray(mask)
    if emissions.shape != (B, S, T) or not mask.all():
        return _run_host(emissions, transitions, tags, mask)
    return _run_device(emissions, transitions, tags)


# revision 24
# speedup vs baseline: 1.6036x; 1.0311x over previous
"""Trainium2 Bass kernel for a batched linear-chain CRF negative log-likelihood.

reference semantics (B=128, S=2048, T=128):
    forward algorithm over S steps (log-space matvec chain) -> log_Z per batch
    gold path score = emissions gathered at tags + transitions gathered at
    (tag_t, tag_{t+1}) pairs, summed over time
    output = mean(log_Z - seq_score)   (scalar f32)

Strategy:
  - data parallel over 8 cores: 16 batch rows per core, transitions replicated.
  - linear space: a_t = (a_{t-1} @ W) * E_t with W = exp(transitions),
    E_t = exp(emit_t - chat).  Per-step work: one PE matmul (stationary W,
    moving state [128 tags x 16 batch]) + one DVE multiply out of PSUM.
  - bidirectional: forward chain from t=0 and a backward chain
    y_t = E_t * (W @ y_{t+1}) from t=2047 run concurrently and meet at
    t=1023: log_Z = log(a_m . (W y_{m+1})) + accumulated log scales.
  - renormalization every 32 steps; colsum scale logs parked and ln'd once
    in the epilogue.
  - E precomputed in a pre-phase into a transposed [tag, b*S+t] bf16 buffer
    via PE transpose + scalar-engine exp evacuation (bias = -chat).
  - gold path in the same pre-phase, via one fp32 matmul per (b, sblock):
    CD_b += OH^T @ [OHshift | EMIS]  (N=256).  The left half accumulates the
    tag-pair count matrix, the right half accumulates D[i,j] = sum_s
    OH[s,i] e[s,j] whose diagonal is the emission-select sum.  Finalized per
    batch row with one elementwise multiply by [trans | identity] and a
    grouped reduce.
"""

import numpy as np

B, S, T = 128, 2048, 128
NCORES = 8
BC = B // NCORES  # 16 batch rows per core
NSB = S // 128  # 16 s-blocks of 128
MID = S // 2 - 1  # 1023: chains meet here
RENORM = 64
JUNK_TAG = 60000.0  # one-hot of this is all zeros (tags are < 128)

_compiled = None


def _build_program(do_chain=True, do_gold=True, nrot=None):
    import concourse.bass as bass
    import concourse.bacc as bacc
    import concourse.tile as tile
    from concourse import mybir
    from concourse.masks import make_identity

    fp32 = mybir.dt.float32
    bf16 = mybir.dt.bfloat16
    AF = mybir.ActivationFunctionType
    ALU = mybir.AluOpType
    AX = mybir.AxisListType

    nc = bacc.Bacc(None)
    em_d = nc.declare_dram_parameter("emissions_sh", [BC, S, T], fp32, isOutput=False)
    tr_d = nc.declare_dram_parameter("transitions", [T, T], fp32, isOutput=False)
    tg_d = nc.declare_dram_parameter("tags_sh", [BC, S], mybir.dt.int32, isOutput=False)
    out_d = nc.declare_dram_parameter("loss_parts", [BC], fp32, isOutput=True)

    with tile.TileContext(nc) as tc:
        with (
            tc.tile_pool(name="consts", bufs=1) as consts,
            tc.tile_pool(name="ebuf", bufs=1) as ebufp,
            tc.tile_pool(name="emis", bufs=8) as emisp,
            tc.tile_pool(name="oh", bufs=8) as ohp,
            tc.tile_pool(name="dump", bufs=6) as dumpp,
            tc.tile_pool(name="state", bufs=8) as statep,
            tc.tile_pool(name="small", bufs=6) as smallp,
            tc.tile_pool(name="tp_ps", bufs=2, space="PSUM") as tp_ps,
            tc.tile_pool(name="q_ps", bufs=4, space="PSUM") as q_ps,
            tc.tile_pool(name="cd_ps", bufs=1, space="PSUM") as cd_ps,
            tc.tile_pool(name="m_ps", bufs=1, space="PSUM") as m_ps,
        ):
            # ---------------- constants ----------------
            ident = consts.tile([128, 128], fp32)
            make_identity(nc, ident)
            ident_bf = consts.tile([128, 128], bf16)
            make_identity(nc, ident_bf)
            iota = consts.tile([128, 128], bf16)
            nc.gpsimd.iota(
                iota, pattern=[[1, 128]], base=0, channel_multiplier=0,
                allow_small_or_imprecise_dtypes=True,
            )
            ones_col_bf = consts.tile([128, 1], bf16)
            nc.vector.memset(ones_col_bf, 1.0)
            ones_col_f = consts.tile([128, 1], fp32)
            nc.vector.memset(ones_col_f, 1.0)
            ones_row_f = consts.tile([1, 128], fp32)
            nc.vector.memset(ones_row_f, 1.0)

            # transitions -> W = exp(trans) bf16, WT = W^T bf16
            tr_sb = consts.tile([128, 128], fp32)
            nc.sync.dma_start(out=tr_sb, in_=tr_d[:, :])
            w_bf = consts.tile([128, 128], bf16)
            nc.scalar.activation(w_bf, tr_sb, AF.Exp)
            wt_psum = tp_ps.tile([128, 128], bf16, tag="tp")
            nc.tensor.transpose(wt_psum, w_bf, ident_bf)
            wt_bf = consts.tile([128, 128], bf16)
            nc.vector.tensor_copy(wt_bf, wt_psum)

            # [trans | identity] for the gold finalize
            tri = consts.tile([128, 256], fp32)
            nc.vector.tensor_copy(tri[:, 0:128], tr_sb)
            nc.vector.tensor_copy(tri[:, 128:256], ident)

            # chat = mean_j ln(colsum_j W) over j=1..127  (col 0 is exp(-1e4)=0)
            colw_ps = m_ps.tile([1, 128], fp32, tag="m")
            nc.tensor.matmul(colw_ps, ones_col_bf, w_bf, start=True, stop=True)
            lncol = smallp.tile([1, 127], fp32, tag="lncol")
            lnsum = consts.tile([1, 1], fp32)
            nc.scalar.activation(lncol, colw_ps[:, 1:128], AF.Ln, accum_out=lnsum)
            chat_tot = consts.tile([1, 1], fp32)
            nc.scalar.activation(chat_tot, lnsum, AF.Copy, scale=float(S) / 127.0)
            negchat = consts.tile([1, 1], fp32)
            nc.scalar.activation(negchat, lnsum, AF.Copy, scale=-1.0 / 127.0)
            nbc_ps = m_ps.tile([128, 1], fp32, tag="m")
            nc.tensor.matmul(nbc_ps, ones_row_f, negchat, start=True, stop=True)
            negchat_bc = consts.tile([128, 1], fp32)
            nc.vector.tensor_copy(negchat_bc, nbc_ps)

            # tags -> f32, transposed into [s(128), (sb,b)] column layout,
            # plus a shift-by-one variant for transition pairs
            tags_nat = consts.tile([BC, S], mybir.dt.int32)
            nc.sync.dma_start(out=tags_nat, in_=tg_d[:, :])
            tags_f = consts.tile([BC, S], fp32)
            nc.vector.tensor_copy(tags_f, tags_nat)
            tag_cols = consts.tile([128, NSB * BC], fp32)   # col = sb*16 + b
            tagsh_cols = consts.tile([128, NSB * BC], fp32)
            nc.vector.memset(tagsh_cols[:, (NSB - 1) * BC:], JUNK_TAG)
            for sb in range(NSB):
                tp = tp_ps.tile([128, BC], fp32, tag="tp")
                nc.tensor.transpose(
                    tp, tags_f[:, sb * 128:(sb + 1) * 128], ident[:BC, :BC]
                )
                nc.vector.tensor_copy(tag_cols[:, sb * BC:(sb + 1) * BC], tp)
            for sb in range(NSB):
                n = 128 if sb < NSB - 1 else 127
                tp = tp_ps.tile([128, BC], fp32, tag="tp")
                nc.tensor.transpose(
                    tp[:n], tags_f[:, sb * 128 + 1: sb * 128 + 1 + n],
                    ident[:BC, :BC],
                )
                nc.vector.tensor_copy(
                    tagsh_cols[:n, sb * BC:(sb + 1) * BC], tp[:n]
                )

            # ---------------- pre-phase: gold + E precompute ----------------
            ebuf = ebufp.tile([128, S * BC], bf16)  # free index = b*S + t
            ebuf3 = ebuf.rearrange("p (b t) -> p b t", t=S)
            # per-b [sum(C*trans) | esel] results: cols [2b, 2b+1]
            gsum = consts.tile([128, 2 * BC], fp32)

            def emit_E(b, sb):
                emis = emisp.tile([128, 128], fp32, tag="emis")
                nc.sync.dma_start(
                    out=emis, in_=em_d[b, sb * 128:(sb + 1) * 128, :]
                )
                tp = tp_ps.tile([128, 128], fp32, tag="tp")
                nc.tensor.transpose(tp, emis, ident)
                # exp(x - chat), contiguous run: free = b*S + sb*128 + s
                nc.scalar.activation(
                    ebuf3[:, b, sb * 128:(sb + 1) * 128], tp, AF.Exp,
                    bias=negchat_bc,
                )

            gold_cd = [None]

            def emit_gold(b, sb):
                col = sb * BC + b
                oh = ohp.tile([128, 128], bf16, tag="oh")
                nc.vector.tensor_scalar(
                    out=oh, in0=iota, scalar1=tag_cols[:, col:col + 1],
                    scalar2=None, op0=ALU.is_equal,
                )
                # rhs = [OHshift | EMIS]
                pair = ohp.tile([128, 256], bf16, tag="pair")
                nc.vector.tensor_scalar(
                    out=pair[:, 0:128], in0=iota,
                    scalar1=tagsh_cols[:, col:col + 1],
                    scalar2=None, op0=ALU.is_equal,
                )
                emis2 = emisp.tile([128, 128], fp32, tag="emis2")
                nc.sync.dma_start(
                    out=emis2, in_=em_d[b, sb * 128:(sb + 1) * 128, :]
                )
                nc.scalar.activation(pair[:, 128:256], emis2, AF.Copy)
                if sb == 0:
                    gold_cd[0] = cd_ps.tile(
                        [128, 256], fp32, tag="cd", name="gold_cd"
                    )
                nc.tensor.matmul(
                    gold_cd[0], oh, pair, start=(sb == 0), stop=(sb == NSB - 1)
                )
                if sb == NSB - 1:
                    # finalize row b: [C|D] * [trans|ident], grouped reduce
                    cdump = dumpp.tile([128, 256], fp32, tag="cdump")
                    nc.vector.tensor_tensor(
                        out=cdump, in0=gold_cd[0], in1=tri, op=ALU.mult
                    )
                    nc.vector.tensor_reduce(
                        gsum[:, 2 * b:2 * b + 2],
                        cdump.rearrange("p (c j) -> p c j", c=2),
                        axis=AX.X, op=ALU.add,
                    )

            side = []
            order = [0, NSB - 1]
            for k in range(1, NSB // 2):
                order += [k, NSB - 1 - k]
            for sb in order[2:]:
                for b in range(BC):
                    side.append(("E", b, sb))
            if do_gold:
                for b in range(BC):
                    for sb in range(NSB):
                        side.append(("G", b, sb))
            else:
                nc.vector.memset(gsum, 0.0)
            for sb in order[:2]:
                for b in range(BC):
                    emit_E(b, sb)

            def do_side(n):
                for _ in range(n):
                    if side:
                        kind, b, sb = side.pop(0)
                        if kind == "E":
                            emit_E(b, sb)
                        else:
                            emit_gold(b, sb)

            # ---------------- chain ----------------
            NRE = 64
            glog = consts.tile([1, BC * NRE], fp32)
            nc.vector.memset(glog, 1.0)
            glog3 = glog.rearrange("p (b k) -> p b k", k=NRE)
            renorm_k = [0]

            def renorm(v):
                """colsum -> reciprocal -> broadcast; park colsum for epilogue."""
                cs = m_ps.tile([1, BC], fp32, tag="m")
                nc.tensor.matmul(cs, ones_col_bf, v, start=True, stop=True)
                rec = smallp.tile([1, BC], fp32, tag="rec")
                nc.vector.reciprocal(rec, cs)
                k = renorm_k[0]
                renorm_k[0] += 1
                nc.vector.tensor_copy(glog3[:, :, k], cs)
                bc_ps = m_ps.tile([128, BC], fp32, tag="m")
                nc.tensor.matmul(bc_ps, ones_row_f, rec, start=True, stop=True)
                return bc_ps

            def eslice(t):
                return ebuf3[:, :, t]

            vf = eslice(0)          # a_0 = E_0
            vb = eslice(S - 1)      # y_{2047} = E_{2047}
            bc_f = None
            bc_b = None
            vb_fin = None
            NROT = S - 1 - MID      # 1024 rotations
            nrot_lim = NROT if nrot is None else nrot
            for r in range(NROT if do_chain else 0):
                if r >= nrot_lim:
                    break
                # forward step t = r+1:  a_t = (a_{t-1} @ W) * E_t  (lhsT=W)
                if r < MID:
                    t = r + 1
                    qf = q_ps.tile([128, BC], fp32, tag="q")
                    nc.tensor.matmul(qf, w_bf, vf, start=True, stop=True)
                    nvf = statep.tile([128, BC], bf16, tag="vf")
                    nc.vector.tensor_tensor(out=nvf, in0=qf, in1=eslice(t), op=ALU.mult)
                    if bc_f is not None:
                        nc.vector.tensor_tensor(out=nvf, in0=nvf, in1=bc_f, op=ALU.mult)
                        bc_f = None
                    vf = nvf
                    if (t % RENORM == 0 or t == 1008) and t < MID:
                        bc_f = renorm(vf)
                # backward: q = W @ y_{t+1}; t from 2046 down to MID
                t = S - 2 - r
                qb = q_ps.tile([128, BC], fp32, tag="q")
                nc.tensor.matmul(qb, wt_bf, vb, start=True, stop=True)
                if t == MID:
                    vb_fin = qb  # b_MID = W y_{MID+1}: final, stays in PSUM
                else:
                    nvb = statep.tile([128, BC], bf16, tag="vb")
                    nc.vector.tensor_tensor(out=nvb, in0=qb, in1=eslice(t), op=ALU.mult)
                    if bc_b is not None:
                        nc.vector.tensor_tensor(out=nvb, in0=nvb, in1=bc_b, op=ALU.mult)
                        bc_b = None
                    vb = nvb
                    # scale from a renorm at t applies at step t-1; last chance
                    # is t == MID+2
                    if (t % RENORM == 0 or t == 1040) and t > MID + 1:
                        bc_b = renorm(vb)
                if (r + 1) % RENORM == 0 or (r + 2) % RENORM == 0:
                    pass  # keep renorm rotations clean
                elif r % 2 == 0:
                    do_side(1)
                elif r % RENORM == 3:
                    do_side(2)

            do_side(len(side))
            if not do_chain or nrot_lim < NROT:
                vvf = statep.tile([128, BC], bf16, tag="vf")
                nc.vector.memset(vvf, 1.0)
                vf = vvf
                vb_fin = q_ps.tile([128, BC], fp32, tag="q", name="vbfin")
                nc.tensor.matmul(vb_fin, wt_bf, vvf, start=True, stop=True)

            # ---------------- epilogue ----------------
            # log_Z = ln(sum_j vf*vb_fin) + sum(ln renorm scales) + S*chat
            dotd = dumpp.tile([128, BC], fp32, tag="dotd")
            nc.vector.tensor_tensor(out=dotd, in0=vb_fin, in1=vf, op=ALU.mult)
            zs = m_ps.tile([1, BC], fp32, tag="m")
            nc.tensor.matmul(zs, ones_col_f, dotd, start=True, stop=True)
            lnz = smallp.tile([1, BC], fp32, tag="lnz")
            nc.scalar.activation(lnz, zs, AF.Ln)
            lnglog = smallp.tile([1, BC * NRE], fp32, tag="lnglog")
            nc.scalar.activation(lnglog, glog, AF.Ln)
            accsum = smallp.tile([1, BC], fp32, tag="accsum")
            nc.vector.tensor_reduce(
                accsum,
                lnglog.rearrange("p (b k) -> p b k", k=NRE),
                axis=AX.X, op=ALU.add,
            )
            logz = smallp.tile([1, BC], fp32, tag="logz")
            nc.vector.tensor_tensor(out=logz, in0=lnz, in1=accsum, op=ALU.add)
            nc.vector.tensor_scalar(
                out=logz, in0=logz, scalar1=chat_tot, scalar2=None, op0=ALU.add
            )

            # seq score from gsum columns: [2b] = sum(C*trans), [2b+1] = esel
            gs_ps = m_ps.tile([1, 2 * BC], fp32, tag="m")
            nc.tensor.matmul(gs_ps, ones_col_f, gsum, start=True, stop=True)
            res = smallp.tile([1, BC], fp32, tag="res")
            seq = gs_ps.rearrange("p (b c) -> p b c", c=2)
            nc.vector.tensor_tensor(out=res, in0=logz, in1=seq[:, :, 0], op=ALU.subtract)
            nc.vector.tensor_tensor(out=res, in0=res, in1=seq[:, :, 1], op=ALU.subtract)
            nc.sync.dma_start(out=out_d[:], in_=res[0:1, :])

    return nc


def _get_compiled(finalized=False):
    global _compiled
    if _compiled is None:
        _compiled = _build_program()
    if finalized and not _compiled.is_finalized():
        _compiled.finalize()
    return _compiled


def make_in_maps(emissions, transitions, tags):
    in_maps = []
    for c in range(NCORES):
        sl = slice(c * BC, (c + 1) * BC)
        in_maps.append({
            "emissions_sh": np.ascontiguousarray(emissions[sl], dtype=np.float32),
            "transitions": np.ascontiguousarray(transitions, dtype=np.float32),
            "tags_sh": np.ascontiguousarray(tags[sl]).astype(np.int32),
        })
    return in_maps


def _run_device(emissions, transitions, tags):
    from concourse.bass_utils import run_bass_kernel_spmd

    nc = _get_compiled(finalized=True)
    res = run_bass_kernel_spmd(
        nc, make_in_maps(emissions, transitions, tags), list(range(NCORES))
    )
    parts = np.concatenate([res.results[c]["loss_parts"] for c in range(NCORES)])
    return np.float32(parts.mean())


def _run_host(emissions, transitions, tags, mask):
    """Slow but fully general fallback (any mask pattern)."""
    e = emissions.astype(np.float64)
    t = transitions.astype(np.float64)

    def lse(x, axis):
        m = x.max(axis=axis, keepdims=True)
        return (m + np.log(np.exp(x - m).sum(axis=axis, keepdims=True))).squeeze(axis)

    score = e[:, 0]
    for s in range(1, e.shape[1]):
        nxt = lse(score[:, :, None] + t[None, :, :] + e[:, s, None, :], axis=1)
        score = np.where(mask[:, s, None], nxt, score)
    log_Z = lse(score, axis=1)
    emit = np.take_along_axis(e, tags[..., None].astype(np.int64), axis=2)[..., 0]
    trans_sc = t[tags[:, :-1].astype(np.int64), tags[:, 1:].astype(np.int64)]
    m = mask[:, 1:].astype(np.float64)
    seq = emit[:, 0] + ((trans_sc + emit[:, 1:]) * m).sum(axis=1)
    return np.float32((log_Z - seq).mean())


def kernel(emissions, transitions, tags, mask):
    emissions = np.asarray(emissions)
    transitions = np.asarray(transitions)
    tags = np.asarray(tags)
    mask = np.asarray(mask)
    if emissions.shape != (B, S, T) or not mask.all():
        return _run_host(emissions, transitions, tags, mask)
    return _run_device(emissions, transitions, tags)
